# revision 2
# baseline (speedup 1.0000x reference)
"""Block-sparse attention Trainium2 kernel (v3 — single-core full problem).

Empirical axon dispatch cost model (measured in this environment):
  - 8-core sharded call: ~3.6ms floor + ~0.4ms/MB/core above ~6MB/core
    (the baseline 8-way kernel paid ~6ms of pure dispatch tax).
  - 1-core call: ~1.1ms floor, input/output bytes FREE up to 68MB, and
    transport overlaps execution; consecutive executes serialize.
  So the whole problem runs on ONE core: per-call ~= exec + ~0.45ms.

Structure: NCHUNK=8 chunks of (8 blocks x 129 tokens) x 4 batches on core
0. Math per chunk identical to the 8-core v2 kernel (bf16 matmuls, fp32
PSUM, transposed score layout, ones-matmul softmax denominators,
PE-broadcast reciprocal normalization). Block 4's attention is emitted
before half-0's output projection so the PE queue has independent work
while the last block's DVE normalization drains.
"""

import numpy as np

H, BLK, DK, DV = 8, 129, 64, 64
B, N, D = 4, 8256, 1024
INNER = H * DK           # 512
NB = N // BLK            # 64 blocks
NCHUNK = 8               # chunks per instance (8 = whole problem)
NINST = 8 // NCHUNK      # kernel instances (cores used)
NBC = 8                  # blocks per chunk
T = NBC * BLK            # 1032 tokens per chunk per batch
TC = NCHUNK * T          # tokens per instance per batch
DC = D // 128            # 8 contraction chunks over D
FC = INNER // 128        # 4 chunks over the 512 inner dim
TSL = [(0, 512), (512, 512), (1024, T - 1024)]

_NC_CACHE = {}


def _weight_layouts(Wq, Wk, Wv, Wo, bo):
    """Precompute the SBUF-layout weight constants (bf16/fp32 numpy)."""
    import ml_dtypes
    bf16 = ml_dtypes.bfloat16
    Wq = np.asarray(Wq, np.float32)
    Wk = np.asarray(Wk, np.float32)
    Wv = np.asarray(Wv, np.float32)
    Wo = np.asarray(Wo, np.float32)
    bo = np.asarray(bo, np.float32)

    # Wq/Wk interleaved so head h lives at (chunk h%4, partition 64*(h//4)):
    # w_sb[p, c, m*128 + 64*a + d] = W[c*128 + p, a*256 + m*64 + d]
    def qk_layout(w):
        w5 = w.reshape(DC, 128, 2, 4, 64)          # [c, p, a, m, d]
        return np.ascontiguousarray(
            w5.transpose(1, 0, 3, 2, 4).reshape(128, DC, INNER)).astype(bf16)

    wq_sb = qk_layout(Wq)
    wk_sb = qk_layout(Wk)
    # Wv standard: w_sb[p, c, f] = Wv[c*128 + p, f]
    wv_sb = np.ascontiguousarray(
        Wv.reshape(DC, 128, INNER).transpose(1, 0, 2)).astype(bf16)
    # Wo: w_sb[p, c, f] = Wo[c*128 + p, f]   (c over INNER chunks)
    wo_sb = np.ascontiguousarray(
        Wo.reshape(FC, 128, D).transpose(1, 0, 2)).astype(bf16)
    # bias column per output-feature chunk: bo_col[p, fch] = bo[fch*128+p]
    bo_col = np.ascontiguousarray(bo.reshape(8, 128).T).astype(np.float32)
    return wq_sb, wk_sb, wv_sb, wo_sb, bo_col


def _build_nc(Wq, Wk, Wv, Wo, bo):
    import concourse.bacc as bacc
    import concourse.tile as tile
    from concourse import mybir

    f32 = mybir.dt.float32
    bf16 = mybir.dt.bfloat16
    f16 = mybir.dt.float16

    wq_np, wk_np, wv_np, wo_np, bo_np = _weight_layouts(Wq, Wk, Wv, Wo, bo)

    nc = bacc.Bacc("TRN2", target_bir_lowering=False, debug=False,
                   num_devices=max(NINST, 1))

    # One input / one output operand (operands cost ~1.3ms apiece on the
    # dispatch path).
    xt = nc.dram_tensor("xtc", [B, 128, DC, TC + NB], bf16,
                        kind="ExternalInput").ap()
    y = nc.dram_tensor("y", [B, 128, DC, TC], f16, kind="ExternalOutput").ap()

    wq_d = nc.inline_tensor(wq_np, name="wq_c").ap()
    wk_d = nc.inline_tensor(wk_np, name="wk_c").ap()
    wv_d = nc.inline_tensor(wv_np, name="wv_c").ap()
    wo_d = nc.inline_tensor(wo_np, name="wo_c").ap()
    bo_d = nc.inline_tensor(bo_np, name="bo_c").ap()

    ex = mybir.ActivationFunctionType.Exp
    idf = mybir.ActivationFunctionType.Identity

    with tile.TileContext(nc) as tc:
        with (
            tc.tile_pool(name="const", bufs=1) as const,
            tc.tile_pool(name="gb", bufs=2) as gb,
            tc.tile_pool(name="single", bufs=1) as sg1,
            tc.tile_pool(name="chunk", bufs=2) as cp,
            tc.tile_pool(name="att", bufs=4) as ap_,
            tc.tile_pool(name="ppsum", bufs=8, space="PSUM") as pp,
        ):
            stp = ogp = smp = pp
            # ---- constants ----
            ones_col = const.tile([128, 1], bf16)
            nc.vector.memset(ones_col, 1.0)
            ones_row = const.tile([1, 128], bf16)
            nc.vector.memset(ones_row, 1.0)
            ones_sq = const.tile([128, 64], bf16)
            nc.vector.memset(ones_sq, 1.0)
            wq_sb = const.tile([128, DC, INNER], bf16)
            wk_sb = const.tile([128, DC, INNER], bf16)
            wv_sb = const.tile([128, DC, INNER], bf16)
            wo_sb = const.tile([128, FC, D], bf16)
            bo_col = const.tile([128, DC], f32)
            nc.sync.dma_start(out=wk_sb, in_=wk_d)
            # global tokens x^T for all batches, loaded once
            xgT = sg1.tile([128, B, DC, NB], bf16, tag="xgT")
            for bb in range(B):
                nc.sync.dma_start(
                    out=xgT[:, bb, :, :], in_=xt[bb, :, :, TC:TC + NB])
            nc.sync.dma_start(out=wq_sb, in_=wq_d)
            nc.sync.dma_start(out=wv_sb, in_=wv_d)

            for b in range(B):
                # ---- global tokens: kgT, vg (per batch) ----
                kgT = gb.tile([128, FC, NB], bf16, tag="kgT")
                for mc in range(FC):
                    pt = smp.tile([128, NB], f32, tag="pp")
                    for dc in range(DC):
                        nc.tensor.matmul(
                            pt, wk_sb[:, dc, mc * 128:(mc + 1) * 128],
                            xgT[:, b, dc, :],
                            start=(dc == 0), stop=(dc == DC - 1))
                    nc.scalar.copy(out=kgT[:, mc, :], in_=pt)
                vg = gb.tile([64, INNER], bf16, tag="vg")
                pt = pp.tile([128, 512], f32, tag="pp")
                for dc in range(DC):
                    nc.tensor.matmul(pt[:64, :], xgT[:, b, dc, 0:64],
                                     wv_sb[:, dc, :],
                                     start=(dc == 0), stop=(dc == DC - 1))
                nc.scalar.copy(out=vg, in_=pt[:64, :])

                for ch in range(NCHUNK):
                    col0 = ch * T

                    # ---- load x^T for this chunk ----
                    xT = cp.tile([128, DC, T], bf16, tag="xT")
                    nc.sync.dma_start(out=xT, in_=xt[b, :, :, col0:col0 + T])

                    # ---- q/k projections (transposed layout) ----
                    qT = cp.tile([128, FC, T], bf16, tag="qT")
                    kT = cp.tile([128, FC, T], bf16, tag="kT")
                    for dst, w_sb, eng in ((qT, wq_sb, "act"),
                                           (kT, wk_sb, "dve")):
                        for mc in range(FC):
                            for t0, tsz in TSL:
                                pt = pp.tile([128, 512], f32, tag="pp")
                                for dc in range(DC):
                                    nc.tensor.matmul(
                                        pt[:, :tsz],
                                        w_sb[:, dc, mc * 128:(mc + 1) * 128],
                                        xT[:, dc, t0:t0 + tsz],
                                        start=(dc == 0), stop=(dc == DC - 1))
                                if eng == "act":
                                    nc.scalar.copy(
                                        out=dst[:, mc, t0:t0 + tsz],
                                        in_=pt[:, :tsz])
                                else:
                                    nc.vector.tensor_copy(
                                        out=dst[:, mc, t0:t0 + tsz],
                                        in_=pt[:, :tsz])

                    # ---- v projection (token-on-partition, per block) ----
                    v = cp.tile([128, NBC, INNER], bf16, tag="v")

                    def v_group(n, xT=xT, v=v):
                        pt = pp.tile([128, 512], f32, tag="pp")
                        for dc in range(DC):
                            nc.tensor.matmul(
                                pt, xT[:, dc, n * BLK:n * BLK + 128],
                                wv_sb[:, dc, :],
                                start=(dc == 0), stop=(dc == DC - 1))
                        nc.vector.tensor_copy(out=v[:, n, :], in_=pt)

                    v_group(0)
                    # last token of each block, batched: tokens 129n+128
                    vl8 = cp.tile([NBC, INNER], bf16, tag="vl8")
                    pt = pp.tile([128, 512], f32, tag="pp")
                    for dc in range(DC):
                        nc.tensor.matmul(pt[:NBC, :], xT[:, dc, 128::BLK],
                                         wv_sb[:, dc, :],
                                         start=(dc == 0), stop=(dc == DC - 1))
                    nc.vector.tensor_copy(out=vl8, in_=pt[:NBC, :])
                    vl_all4 = cp.tile([128, NBC, INNER], bf16, tag="vlall")
                    for hp in range(4):
                        nc.sync.dma_start(out=vl_all4[32 * hp:32 * hp + 1],
                                          in_=vl8)

                    outT = cp.tile([128, FC, T], bf16, tag="outT")
                    if b == 0 and ch == 0:
                        nc.sync.dma_start(out=wo_sb, in_=wo_d)
                        nc.sync.dma_start(out=bo_col, in_=bo_d)

                    # ---- global attention for this chunk's 8 blocks ----
                    eg = cp.tile([64, H, NBC], bf16, tag="eg")
                    lg = smp.tile([1, H * NBC], f32, tag="pp")
                    sgt = smp.tile([64, H, NBC], f32, tag="pp")
                    for h in range(H):
                        p0 = 64 * (h // 4)
                        hc = h % 4
                        nc.tensor.matmul(sgt[:, h, :], kgT[p0:p0 + 64, hc, :],
                                         qT[p0:p0 + 64, hc, 0::BLK],
                                         start=True, stop=True)
                    nc.scalar.activation(out=eg, in_=sgt, func=ex, scale=0.125)
                    nc.tensor.matmul(lg, ones_col[0:64, :], eg,
                                     start=True, stop=True)
                    rlg = cp.tile([1, H * NBC], bf16, tag="rlg")
                    with nc.allow_low_precision("1/l to bf16"):
                        nc.vector.reciprocal(out=rlg, in_=lg)
                    ogn = cp.tile([128, FC, NBC], bf16, tag="ogn")
                    for hp in range(4):
                        ogg = smp.tile([128, NBC], f32, tag="pp")
                        for hh in range(2):
                            h = 2 * hp + hh
                            nc.tensor.matmul(
                                ogg[64 * hh:64 * hh + 64, :],
                                vg[:, h * DV:(h + 1) * DV], eg[:, h, :],
                                start=True, stop=True)
                        rlbg = smp.tile([128, NBC], f32, tag="pp")
                        for hh in range(2):
                            o0 = hp * 2 * NBC + hh * NBC
                            nc.tensor.matmul(
                                rlbg[64 * hh:64 * hh + 64, :],
                                ones_row[0:1, 0:64],
                                rlg[0:1, o0:o0 + NBC],
                                start=True, stop=True)
                        rlbg_sb = cp.tile([128, NBC], bf16, tag="rlbg_sb")
                        nc.scalar.copy(out=rlbg_sb, in_=rlbg)
                        nc.vector.tensor_mul(out=ogn[:, hp, :], in0=ogg,
                                             in1=rlbg_sb)

                    # ---- block-local attention ----
                    def attn_block(n):
                        c0 = n * BLK
                        eT = ap_.tile([128, H, BLK], bf16, tag="eT")
                        eTl = ap_.tile([128, 2 * BLK], bf16, tag="eTl")
                        rl = ap_.tile([128, 2 * BLK], bf16, tag="rl")
                        stl = smp.tile([128, 2 * BLK], f32, tag="pp")
                        lp = smp.tile([128, 2 * BLK], f32, tag="pp")
                        for hp in range(4):
                            st = stp.tile([128, 2 * BLK], f32, tag="pp")
                            r0 = 32 * hp
                            for hh in range(2):
                                h = 2 * hp + hh
                                p0 = 64 * (h // 4)
                                hc = h % 4
                                lq = qT[p0:p0 + 64, hc, c0:c0 + BLK]
                                nc.tensor.matmul(
                                    st[:, hh * BLK:(hh + 1) * BLK],
                                    kT[p0:p0 + 64, hc, c0:c0 + 128], lq,
                                    start=True, stop=True)
                                nc.tensor.matmul(
                                    stl[r0:r0 + 1, hh * BLK:(hh + 1) * BLK],
                                    kT[p0:p0 + 64, hc, c0 + 128:c0 + BLK],
                                    lq, start=True, stop=True,
                                    tile_position=(p0, r0))
                            nc.scalar.activation(
                                out=eT[:, 2 * hp:2 * hp + 2, :], in_=st,
                                func=ex, scale=0.125)
                        nc.scalar.activation(
                            out=eTl[0:97, :], in_=stl[0:97, :],
                            func=ex, scale=0.125)
                        for hp in range(4):
                            r0 = 32 * hp
                            nc.tensor.matmul(lp[r0:r0 + 1, :], ones_col,
                                             eT[:, 2 * hp:2 * hp + 2, :],
                                             start=True, stop=True,
                                             tile_position=(0, r0))
                        nc.vector.tensor_add(
                            out=lp[0:97, :], in0=lp[0:97, :],
                            in1=eTl[0:97, :])
                        with nc.allow_low_precision("1/l to bf16"):
                            nc.vector.reciprocal(
                                out=rl[0:97, :], in_=lp[0:97, :])
                        for hp in range(4):
                            r0 = 32 * hp
                            og = ogp.tile([128, BLK], f32, tag="pp")
                            for hh in range(2):
                                h = 2 * hp + hh
                                nc.tensor.matmul(
                                    og[64 * hh:64 * hh + 64, :],
                                    v[:, n, h * DV:(h + 1) * DV],
                                    eT[:, h, :], start=True, stop=False)
                                nc.tensor.matmul(
                                    og[64 * hh:64 * hh + 64, :],
                                    vl_all4[r0:r0 + 1, n,
                                            h * DV:(h + 1) * DV],
                                    eTl[r0:r0 + 1,
                                        hh * BLK:(hh + 1) * BLK],
                                    start=False, stop=True,
                                    tile_position=(r0, 64 * hh))
                            rlb = ogp.tile([128, BLK], f32, tag="pp")
                            for hh in range(2):
                                nc.tensor.matmul(
                                    rlb[64 * hh:64 * hh + 64, :],
                                    ones_sq[r0:r0 + 1, 0:64],
                                    rl[r0:r0 + 1,
                                       hh * BLK:(hh + 1) * BLK],
                                    start=True, stop=True,
                                    tile_position=(r0, 64 * hh))
                            rlb_sb = ap_.tile([128, BLK], bf16,
                                              tag="rlb_sb")
                            if hp % 2 == 0:
                                nc.scalar.copy(out=rlb_sb, in_=rlb)
                            else:
                                nc.vector.tensor_copy(out=rlb_sb, in_=rlb)
                            nc.vector.tensor_mul(
                                out=outT[:, hp, c0:c0 + BLK], in0=og,
                                in1=rlb_sb)

                    def slot0_add(half):
                        h0 = 4 * half
                        nc.vector.tensor_add(
                            out=outT[:, :, h0 * BLK:(h0 + 4) * BLK:BLK],
                            in0=outT[:, :, h0 * BLK:(h0 + 4) * BLK:BLK],
                            in1=ogn[:, :, h0:h0 + 4])

                    yT = cp.tile([128, DC, T], f16, tag="yT")

                    def out_proj(half):
                        for t0, tsz in (TSL[0:1] if half == 0 else TSL[1:]):
                            i = 0 if half == 0 else 1
                            for fch in range(DC):
                                yp = pp.tile([128, 512], f32, tag="pp")
                                for fc in range(FC):
                                    nc.tensor.matmul(
                                        yp[:, :tsz],
                                        wo_sb[:, fc,
                                              fch * 128:(fch + 1) * 128],
                                        outT[:, fc, t0:t0 + tsz],
                                        start=(fc == 0), stop=(fc == FC - 1))
                                if (fch + i) % 2 == 0:
                                    nc.scalar.activation(
                                        out=yT[:, fch, t0:t0 + tsz],
                                        in_=yp[:, :tsz], func=idf,
                                        bias=bo_col[:, fch:fch + 1])
                                else:
                                    nc.vector.tensor_scalar_add(
                                        out=yT[:, fch, t0:t0 + tsz],
                                        in0=yp[:, :tsz],
                                        scalar1=bo_col[:, fch:fch + 1])
                        if half == 0:
                            nc.sync.dma_start(
                                out=y[b][:, :, col0:col0 + 512],
                                in_=yT[:, :, 0:512])
                        else:
                            nc.sync.dma_start(
                                out=y[b][:, :, col0 + 512:col0 + T],
                                in_=yT[:, :, 512:T])

                    # blocks 0-4 first: block 4's PE work sits behind
                    # half-0's projection in the queue, covering the DVE
                    # normalization tail of block 3
                    for n in range(5):
                        if n < NBC - 1:
                            v_group(n + 1)
                        attn_block(n)
                        if n == 3:
                            slot0_add(0)
                            out_proj(0)
                    for n in range(5, NBC):
                        if n < NBC - 1:
                            v_group(n + 1)
                        attn_block(n)
                    slot0_add(1)
                    out_proj(1)

    nc.compile()
    return nc


def _key(*arrs):
    import hashlib
    m = hashlib.sha1()
    m.update(f"v3-nchunk{NCHUNK}-r6".encode())
    for a in arrs:
        m.update(np.ascontiguousarray(a, dtype=np.float32).tobytes())
    return m.hexdigest()


def _get_nc(Wq, Wk, Wv, Wo, bo):
    k = _key(Wq, Wk, Wv, Wo, bo)
    if k not in _NC_CACHE:
        _NC_CACHE[k] = _build_nc(Wq, Wk, Wv, Wo, bo)
    return _NC_CACHE[k]


def prep_core_inputs(x):
    """Host-side layout prep: per-instance transposed bf16 activations."""
    import ml_dtypes
    bf16 = ml_dtypes.bfloat16
    x = np.asarray(x, dtype=np.float32)
    xg = x[:, ::BLK, :]                            # [B, NB, D]
    xgt = xg.reshape(B, NB, DC, 128).transpose(0, 3, 2, 1).astype(bf16)
    in_maps = []
    for c in range(NINST):
        xs = x[:, c * TC:(c + 1) * TC, :]          # [B, TC, D]
        xtc = xs.reshape(B, TC, DC, 128).transpose(0, 3, 2, 1).astype(bf16)
        merged = np.ascontiguousarray(
            np.concatenate([xtc, xgt], axis=3))    # [B, 128, DC, TC+NB]
        in_maps.append({"xtc": merged})
    return in_maps


def unpack_output(res_list):
    """[NINST] of y [B, 128, DC, TC] fp16 -> full [B, N, D] fp32."""
    parts = []
    for c in range(NINST):
        yt = np.asarray(res_list[c]).astype(np.float32)  # [B,128,DC,TC]
        parts.append(yt.transpose(0, 3, 2, 1).reshape(B, TC, D))
    return np.concatenate(parts, axis=1)


def kernel(x, Wq, Wk, Wv, Wo, bo):
    from concourse.bass_utils import run_bass_kernel_spmd

    nc = _get_nc(Wq, Wk, Wv, Wo, bo)
    in_maps = prep_core_inputs(x)
    res = run_bass_kernel_spmd(nc, in_maps, core_ids=list(range(NINST)))
    return unpack_output([res.results[c]["y"] for c in range(NINST)])


# revision 3
# speedup vs baseline: 1.0226x; 1.0226x over previous
"""Block-sparse attention Trainium2 kernel (v3 — two-core split).

Empirical axon dispatch cost model (measured in this environment):
  - per-call floor scales with participating cores (1: ~1.1ms, 2: ~2.5ms,
    8: ~3.6ms) plus ~1.3ms per operand plus a super-linear per-core
    input-byte cost (the 8-way baseline paid ~6ms of pure dispatch tax);
  - transport overlaps NEFF execution; executes on a core serialize.
  Measured per-call: 8 cores 6.2ms, 1 core 5.36ms, 2 cores 5.00ms —
  2 cores wins: exec halves (~3.0ms/core) while the 34MB/core input
  staging mostly hides under it.

Structure: each instance runs NCHUNK=4 chunks of (8 blocks x 129 tokens)
x 4 batches. Math per chunk identical to the 8-core v2 kernel (bf16
matmuls, fp32 PSUM, transposed score layout, ones-matmul softmax
denominators, PE-broadcast reciprocal normalization).
"""

import numpy as np

H, BLK, DK, DV = 8, 129, 64, 64
B, N, D = 4, 8256, 1024
INNER = H * DK           # 512
NB = N // BLK            # 64 blocks
NCHUNK = 4               # chunks per instance
NINST = 8 // NCHUNK      # kernel instances (cores used)
NBC = 8                  # blocks per chunk
T = NBC * BLK            # 1032 tokens per chunk per batch
TC = NCHUNK * T          # tokens per instance per batch
DC = D // 128            # 8 contraction chunks over D
FC = INNER // 128        # 4 chunks over the 512 inner dim
TSL = [(0, 512), (512, 512), (1024, T - 1024)]

_NC_CACHE = {}


def _weight_layouts(Wq, Wk, Wv, Wo, bo):
    """Precompute the SBUF-layout weight constants (bf16/fp32 numpy)."""
    import ml_dtypes
    bf16 = ml_dtypes.bfloat16
    Wq = np.asarray(Wq, np.float32)
    Wk = np.asarray(Wk, np.float32)
    Wv = np.asarray(Wv, np.float32)
    Wo = np.asarray(Wo, np.float32)
    bo = np.asarray(bo, np.float32)

    # Wq/Wk interleaved so head h lives at (chunk h%4, partition 64*(h//4)):
    # w_sb[p, c, m*128 + 64*a + d] = W[c*128 + p, a*256 + m*64 + d]
    def qk_layout(w):
        w5 = w.reshape(DC, 128, 2, 4, 64)          # [c, p, a, m, d]
        return np.ascontiguousarray(
            w5.transpose(1, 0, 3, 2, 4).reshape(128, DC, INNER)).astype(bf16)

    wq_sb = qk_layout(Wq)
    wk_sb = qk_layout(Wk)
    # Wv standard: w_sb[p, c, f] = Wv[c*128 + p, f]
    wv_sb = np.ascontiguousarray(
        Wv.reshape(DC, 128, INNER).transpose(1, 0, 2)).astype(bf16)
    # Wo: w_sb[p, c, f] = Wo[c*128 + p, f]   (c over INNER chunks)
    wo_sb = np.ascontiguousarray(
        Wo.reshape(FC, 128, D).transpose(1, 0, 2)).astype(bf16)
    # bias column per output-feature chunk: bo_col[p, fch] = bo[fch*128+p]
    bo_col = np.ascontiguousarray(bo.reshape(8, 128).T).astype(np.float32)
    return wq_sb, wk_sb, wv_sb, wo_sb, bo_col


def _build_nc(Wq, Wk, Wv, Wo, bo):
    import concourse.bacc as bacc
    import concourse.tile as tile
    from concourse import mybir

    f32 = mybir.dt.float32
    bf16 = mybir.dt.bfloat16
    f16 = mybir.dt.float16

    wq_np, wk_np, wv_np, wo_np, bo_np = _weight_layouts(Wq, Wk, Wv, Wo, bo)

    nc = bacc.Bacc("TRN2", target_bir_lowering=False, debug=False,
                   num_devices=max(NINST, 1))

    # One input / one output operand (operands cost ~1.3ms apiece on the
    # dispatch path).
    xt = nc.dram_tensor("xtc", [B, 128, DC, TC + NB], bf16,
                        kind="ExternalInput").ap()
    y = nc.dram_tensor("y", [B, 128, DC, TC], f16, kind="ExternalOutput").ap()

    wq_d = nc.inline_tensor(wq_np, name="wq_c").ap()
    wk_d = nc.inline_tensor(wk_np, name="wk_c").ap()
    wv_d = nc.inline_tensor(wv_np, name="wv_c").ap()
    wo_d = nc.inline_tensor(wo_np, name="wo_c").ap()
    bo_d = nc.inline_tensor(bo_np, name="bo_c").ap()

    ex = mybir.ActivationFunctionType.Exp
    idf = mybir.ActivationFunctionType.Identity

    with tile.TileContext(nc) as tc:
        with (
            tc.tile_pool(name="const", bufs=1) as const,
            tc.tile_pool(name="gb", bufs=2) as gb,
            tc.tile_pool(name="single", bufs=1) as sg1,
            tc.tile_pool(name="chunk", bufs=2) as cp,
            tc.tile_pool(name="att", bufs=4) as ap_,
            tc.tile_pool(name="ppsum", bufs=8, space="PSUM") as pp,
        ):
            stp = ogp = smp = pp
            # ---- constants ----
            ones_col = const.tile([128, 1], bf16)
            nc.vector.memset(ones_col, 1.0)
            ones_row = const.tile([1, 128], bf16)
            nc.vector.memset(ones_row, 1.0)
            ones_sq = const.tile([128, 64], bf16)
            nc.vector.memset(ones_sq, 1.0)
            wq_sb = const.tile([128, DC, INNER], bf16)
            wk_sb = const.tile([128, DC, INNER], bf16)
            wv_sb = const.tile([128, DC, INNER], bf16)
            wo_sb = const.tile([128, FC, D], bf16)
            bo_col = const.tile([128, DC], f32)
            nc.sync.dma_start(out=wk_sb, in_=wk_d)
            # global tokens x^T for all batches, loaded once
            xgT = sg1.tile([128, B, DC, NB], bf16, tag="xgT")
            for bb in range(B):
                nc.sync.dma_start(
                    out=xgT[:, bb, :, :], in_=xt[bb, :, :, TC:TC + NB])
            nc.sync.dma_start(out=wq_sb, in_=wq_d)
            nc.sync.dma_start(out=wv_sb, in_=wv_d)

            for b in range(B):
                # ---- global tokens: kgT, vg (per batch) ----
                kgT = gb.tile([128, FC, NB], bf16, tag="kgT")
                for mc in range(FC):
                    pt = smp.tile([128, NB], f32, tag="pp")
                    for dc in range(DC):
                        nc.tensor.matmul(
                            pt, wk_sb[:, dc, mc * 128:(mc + 1) * 128],
                            xgT[:, b, dc, :],
                            start=(dc == 0), stop=(dc == DC - 1))
                    nc.scalar.copy(out=kgT[:, mc, :], in_=pt)
                vg = gb.tile([64, INNER], bf16, tag="vg")
                pt = pp.tile([128, 512], f32, tag="pp")
                for dc in range(DC):
                    nc.tensor.matmul(pt[:64, :], xgT[:, b, dc, 0:64],
                                     wv_sb[:, dc, :],
                                     start=(dc == 0), stop=(dc == DC - 1))
                nc.scalar.copy(out=vg, in_=pt[:64, :])

                for ch in range(NCHUNK):
                    col0 = ch * T

                    # ---- load x^T for this chunk ----
                    xT = cp.tile([128, DC, T], bf16, tag="xT")
                    nc.sync.dma_start(out=xT, in_=xt[b, :, :, col0:col0 + T])

                    # ---- q/k projections (transposed layout) ----
                    qT = cp.tile([128, FC, T], bf16, tag="qT")
                    kT = cp.tile([128, FC, T], bf16, tag="kT")
                    for dst, w_sb, eng in ((qT, wq_sb, "act"),
                                           (kT, wk_sb, "dve")):
                        for mc in range(FC):
                            for t0, tsz in TSL:
                                pt = pp.tile([128, 512], f32, tag="pp")
                                for dc in range(DC):
                                    nc.tensor.matmul(
                                        pt[:, :tsz],
                                        w_sb[:, dc, mc * 128:(mc + 1) * 128],
                                        xT[:, dc, t0:t0 + tsz],
                                        start=(dc == 0), stop=(dc == DC - 1))
                                if eng == "act":
                                    nc.scalar.copy(
                                        out=dst[:, mc, t0:t0 + tsz],
                                        in_=pt[:, :tsz])
                                else:
                                    nc.vector.tensor_copy(
                                        out=dst[:, mc, t0:t0 + tsz],
                                        in_=pt[:, :tsz])

                    # ---- v projection (token-on-partition, per block) ----
                    v = cp.tile([128, NBC, INNER], bf16, tag="v")

                    def v_group(n, xT=xT, v=v):
                        pt = pp.tile([128, 512], f32, tag="pp")
                        for dc in range(DC):
                            nc.tensor.matmul(
                                pt, xT[:, dc, n * BLK:n * BLK + 128],
                                wv_sb[:, dc, :],
                                start=(dc == 0), stop=(dc == DC - 1))
                        nc.vector.tensor_copy(out=v[:, n, :], in_=pt)

                    v_group(0)
                    # last token of each block, batched: tokens 129n+128
                    vl8 = cp.tile([NBC, INNER], bf16, tag="vl8")
                    pt = pp.tile([128, 512], f32, tag="pp")
                    for dc in range(DC):
                        nc.tensor.matmul(pt[:NBC, :], xT[:, dc, 128::BLK],
                                         wv_sb[:, dc, :],
                                         start=(dc == 0), stop=(dc == DC - 1))
                    nc.vector.tensor_copy(out=vl8, in_=pt[:NBC, :])
                    vl_all4 = cp.tile([128, NBC, INNER], bf16, tag="vlall")
                    for hp in range(4):
                        nc.sync.dma_start(out=vl_all4[32 * hp:32 * hp + 1],
                                          in_=vl8)

                    outT = cp.tile([128, FC, T], bf16, tag="outT")
                    if b == 0 and ch == 0:
                        nc.sync.dma_start(out=wo_sb, in_=wo_d)
                        nc.sync.dma_start(out=bo_col, in_=bo_d)

                    # ---- global attention for this chunk's 8 blocks ----
                    eg = cp.tile([64, H, NBC], bf16, tag="eg")
                    lg = smp.tile([1, H * NBC], f32, tag="pp")
                    sgt = smp.tile([64, H, NBC], f32, tag="pp")
                    for h in range(H):
                        p0 = 64 * (h // 4)
                        hc = h % 4
                        nc.tensor.matmul(sgt[:, h, :], kgT[p0:p0 + 64, hc, :],
                                         qT[p0:p0 + 64, hc, 0::BLK],
                                         start=True, stop=True)
                    nc.scalar.activation(out=eg, in_=sgt, func=ex, scale=0.125)
                    nc.tensor.matmul(lg, ones_col[0:64, :], eg,
                                     start=True, stop=True)
                    rlg = cp.tile([1, H * NBC], bf16, tag="rlg")
                    with nc.allow_low_precision("1/l to bf16"):
                        nc.vector.reciprocal(out=rlg, in_=lg)
                    ogn = cp.tile([128, FC, NBC], bf16, tag="ogn")
                    for hp in range(4):
                        ogg = smp.tile([128, NBC], f32, tag="pp")
                        for hh in range(2):
                            h = 2 * hp + hh
                            nc.tensor.matmul(
                                ogg[64 * hh:64 * hh + 64, :],
                                vg[:, h * DV:(h + 1) * DV], eg[:, h, :],
                                start=True, stop=True)
                        rlbg = smp.tile([128, NBC], f32, tag="pp")
                        for hh in range(2):
                            o0 = hp * 2 * NBC + hh * NBC
                            nc.tensor.matmul(
                                rlbg[64 * hh:64 * hh + 64, :],
                                ones_row[0:1, 0:64],
                                rlg[0:1, o0:o0 + NBC],
                                start=True, stop=True)
                        rlbg_sb = cp.tile([128, NBC], bf16, tag="rlbg_sb")
                        nc.scalar.copy(out=rlbg_sb, in_=rlbg)
                        nc.vector.tensor_mul(out=ogn[:, hp, :], in0=ogg,
                                             in1=rlbg_sb)

                    # ---- block-local attention ----
                    def attn_block(n):
                        c0 = n * BLK
                        eT = ap_.tile([128, H, BLK], bf16, tag="eT")
                        eTl = ap_.tile([128, 2 * BLK], bf16, tag="eTl")
                        rl = ap_.tile([128, 2 * BLK], bf16, tag="rl")
                        stl = smp.tile([128, 2 * BLK], f32, tag="pp")
                        lp = smp.tile([128, 2 * BLK], f32, tag="pp")
                        for hp in range(4):
                            st = stp.tile([128, 2 * BLK], f32, tag="pp")
                            r0 = 32 * hp
                            for hh in range(2):
                                h = 2 * hp + hh
                                p0 = 64 * (h // 4)
                                hc = h % 4
                                lq = qT[p0:p0 + 64, hc, c0:c0 + BLK]
                                nc.tensor.matmul(
                                    st[:, hh * BLK:(hh + 1) * BLK],
                                    kT[p0:p0 + 64, hc, c0:c0 + 128], lq,
                                    start=True, stop=True)
                                nc.tensor.matmul(
                                    stl[r0:r0 + 1, hh * BLK:(hh + 1) * BLK],
                                    kT[p0:p0 + 64, hc, c0 + 128:c0 + BLK],
                                    lq, start=True, stop=True,
                                    tile_position=(p0, r0))
                            nc.scalar.activation(
                                out=eT[:, 2 * hp:2 * hp + 2, :], in_=st,
                                func=ex, scale=0.125)
                        nc.scalar.activation(
                            out=eTl[0:97, :], in_=stl[0:97, :],
                            func=ex, scale=0.125)
                        for hp in range(4):
                            r0 = 32 * hp
                            nc.tensor.matmul(lp[r0:r0 + 1, :], ones_col,
                                             eT[:, 2 * hp:2 * hp + 2, :],
                                             start=True, stop=True,
                                             tile_position=(0, r0))
                        nc.vector.tensor_add(
                            out=lp[0:97, :], in0=lp[0:97, :],
                            in1=eTl[0:97, :])
                        with nc.allow_low_precision("1/l to bf16"):
                            nc.vector.reciprocal(
                                out=rl[0:97, :], in_=lp[0:97, :])
                        for hp in range(4):
                            r0 = 32 * hp
                            og = ogp.tile([128, BLK], f32, tag="pp")
                            for hh in range(2):
                                h = 2 * hp + hh
                                nc.tensor.matmul(
                                    og[64 * hh:64 * hh + 64, :],
                                    v[:, n, h * DV:(h + 1) * DV],
                                    eT[:, h, :], start=True, stop=False)
                                nc.tensor.matmul(
                                    og[64 * hh:64 * hh + 64, :],
                                    vl_all4[r0:r0 + 1, n,
                                            h * DV:(h + 1) * DV],
                                    eTl[r0:r0 + 1,
                                        hh * BLK:(hh + 1) * BLK],
                                    start=False, stop=True,
                                    tile_position=(r0, 64 * hh))
                            rlb = ogp.tile([128, BLK], f32, tag="pp")
                            for hh in range(2):
                                nc.tensor.matmul(
                                    rlb[64 * hh:64 * hh + 64, :],
                                    ones_sq[r0:r0 + 1, 0:64],
                                    rl[r0:r0 + 1,
                                       hh * BLK:(hh + 1) * BLK],
                                    start=True, stop=True,
                                    tile_position=(r0, 64 * hh))
                            rlb_sb = ap_.tile([128, BLK], bf16,
                                              tag="rlb_sb")
                            if hp % 2 == 0:
                                nc.scalar.copy(out=rlb_sb, in_=rlb)
                            else:
                                nc.vector.tensor_copy(out=rlb_sb, in_=rlb)
                            nc.vector.tensor_mul(
                                out=outT[:, hp, c0:c0 + BLK], in0=og,
                                in1=rlb_sb)

                    def slot0_add(half):
                        h0 = 4 * half
                        nc.vector.tensor_add(
                            out=outT[:, :, h0 * BLK:(h0 + 4) * BLK:BLK],
                            in0=outT[:, :, h0 * BLK:(h0 + 4) * BLK:BLK],
                            in1=ogn[:, :, h0:h0 + 4])

                    yT = cp.tile([128, DC, T], f16, tag="yT")

                    def out_proj(half):
                        for t0, tsz in (TSL[0:1] if half == 0 else TSL[1:]):
                            i = 0 if half == 0 else 1
                            for fch in range(DC):
                                yp = pp.tile([128, 512], f32, tag="pp")
                                for fc in range(FC):
                                    nc.tensor.matmul(
                                        yp[:, :tsz],
                                        wo_sb[:, fc,
                                              fch * 128:(fch + 1) * 128],
                                        outT[:, fc, t0:t0 + tsz],
                                        start=(fc == 0), stop=(fc == FC - 1))
                                if (fch + i) % 2 == 0:
                                    nc.scalar.activation(
                                        out=yT[:, fch, t0:t0 + tsz],
                                        in_=yp[:, :tsz], func=idf,
                                        bias=bo_col[:, fch:fch + 1])
                                else:
                                    nc.vector.tensor_scalar_add(
                                        out=yT[:, fch, t0:t0 + tsz],
                                        in0=yp[:, :tsz],
                                        scalar1=bo_col[:, fch:fch + 1])
                        if half == 0:
                            nc.sync.dma_start(
                                out=y[b][:, :, col0:col0 + 512],
                                in_=yT[:, :, 0:512])
                        else:
                            nc.sync.dma_start(
                                out=y[b][:, :, col0 + 512:col0 + T],
                                in_=yT[:, :, 512:T])

                    # blocks 0-4 first: block 4's PE work sits behind
                    # half-0's projection in the queue, covering the DVE
                    # normalization tail of block 3
                    for n in range(5):
                        if n < NBC - 1:
                            v_group(n + 1)
                        attn_block(n)
                        if n == 3:
                            slot0_add(0)
                            out_proj(0)
                    for n in range(5, NBC):
                        if n < NBC - 1:
                            v_group(n + 1)
                        attn_block(n)
                    slot0_add(1)
                    out_proj(1)

    nc.compile()
    return nc


def _key(*arrs):
    import hashlib
    m = hashlib.sha1()
    m.update(f"v3-nchunk{NCHUNK}-r7".encode())
    for a in arrs:
        m.update(np.ascontiguousarray(a, dtype=np.float32).tobytes())
    return m.hexdigest()


def _get_nc(Wq, Wk, Wv, Wo, bo):
    k = _key(Wq, Wk, Wv, Wo, bo)
    if k not in _NC_CACHE:
        _NC_CACHE[k] = _build_nc(Wq, Wk, Wv, Wo, bo)
    return _NC_CACHE[k]


def prep_core_inputs(x):
    """Host-side layout prep: per-instance transposed bf16 activations."""
    import ml_dtypes
    bf16 = ml_dtypes.bfloat16
    x = np.asarray(x, dtype=np.float32)
    xg = x[:, ::BLK, :]                            # [B, NB, D]
    xgt = xg.reshape(B, NB, DC, 128).transpose(0, 3, 2, 1).astype(bf16)
    in_maps = []
    for c in range(NINST):
        xs = x[:, c * TC:(c + 1) * TC, :]          # [B, TC, D]
        xtc = xs.reshape(B, TC, DC, 128).transpose(0, 3, 2, 1).astype(bf16)
        merged = np.ascontiguousarray(
            np.concatenate([xtc, xgt], axis=3))    # [B, 128, DC, TC+NB]
        in_maps.append({"xtc": merged})
    return in_maps


def unpack_output(res_list):
    """[NINST] of y [B, 128, DC, TC] fp16 -> full [B, N, D] fp32."""
    parts = []
    for c in range(NINST):
        yt = np.asarray(res_list[c]).astype(np.float32)  # [B,128,DC,TC]
        parts.append(yt.transpose(0, 3, 2, 1).reshape(B, TC, D))
    return np.concatenate(parts, axis=1)


def kernel(x, Wq, Wk, Wv, Wo, bo):
    from concourse.bass_utils import run_bass_kernel_spmd

    nc = _get_nc(Wq, Wk, Wv, Wo, bo)
    in_maps = prep_core_inputs(x)
    res = run_bass_kernel_spmd(nc, in_maps, core_ids=list(range(NINST)))
    return unpack_output([res.results[c]["y"] for c in range(NINST)])
